# revision 14
# baseline (speedup 1.0000x reference)
"""Bass/Trainium2 kernel for nn_Attention (additive attention + softmax + weighted sum).

Data-parallel over 8 NeuronCores: batch dim (256) sharded 32/core, weights
replicated. Everything below runs per-core on its shard.

Math (per batch item b, X = encoder_out[b] in [196, 1792], dh = decoder_hidden[b]):
  att1 = X @ W_enc + b_enc            [196, 512]
  att2 = dh @ W_dec + b_dec           [512]
  att  = relu(att1 + att2) @ W_full   [196]   (+ b_full, dropped: softmax-invariant)
  alpha = softmax(att)                [196]
  awe  = alpha @ X                    [1792]

Device layout strategy:
  - att1^T [a, t] computed with W_enc tiles stationary and X^T streaming
    (host supplies X^T in bf16; PE contracts over the 1792-dim in 14 k-tiles).
  - att2^T [a, b] computed once per core; folded into the relu evacuation as
    a per-partition ScalarE bias (evacuation is sliced at batch boundaries).
  - score = sum_a relu(...)·W_full via PE with W_full as a [128, 1] stationary
    column, accumulating the 4 a-tiles into a [1, t] PSUM row.
  - softmax on partition-0 rows (scores are in [-2, 2]: no max subtraction).
  - alpha rows transposed back onto partitions with tiny PE transposes, then
    expanded to a block-diagonal [t=128, b=32] lhsT so awe accumulates for all
    32 batch items in 4 persistent PSUM banks across the 49 token-tiles.
  - awe uses X in natural layout (host supplies both layouts in bf16; total
    per-core DMA 45 MB, under the compute roofline).
"""

import numpy as np
import ml_dtypes

import concourse.bass as bass
import concourse.tile as tile
from concourse import bacc, mybir

bf16 = ml_dtypes.bfloat16
AF = mybir.ActivationFunctionType
FP32 = mybir.dt.float32
BF16 = mybir.dt.bfloat16

B, N, ENC, DEC, ATT = 256, 196, 1792, 512, 512
NCORES = 8
B32 = B // NCORES            # 32 batch items per core
T = B32 * N                  # 6272 tokens per core
KT = ENC // 128              # 14 contraction tiles
AT = ATT // 128              # 4 a-tiles
DT = DEC // 128              # 4 d-tiles
CW = 512                     # token-chunk width
NCHUNK = (T + CW - 1) // CW  # 13 (12x512 + 1x128)
TT = T // 128                # 49 token-tiles of 128


def _batch_pieces(lo, hi):
    """Split token range [lo, hi) at batch (196) boundaries -> [(b, t0, t1)]."""
    out = []
    t0 = lo
    while t0 < hi:
        b = t0 // N
        t1 = min(hi, (b + 1) * N)
        out.append((b, t0, t1))
        t0 = t1
    return out


def build(trace_sim=False, nchunks=NCHUNK, level=5, colt=True):
    nc = bacc.Bacc()

    xt_e = nc.declare_dram_parameter("xt", [NCHUNK, 128, KT, CW], BF16, isOutput=False)
    xn_e = nc.declare_dram_parameter("xn", [T, ENC], BF16, isOutput=False)
    we_e = nc.declare_dram_parameter("we", [128, KT, ATT], BF16, isOutput=False)
    wd_e = nc.declare_dram_parameter("wd", [128, DT, ATT], BF16, isOutput=False)
    dht_e = nc.declare_dram_parameter("dht", [128, DT, B32], BF16, isOutput=False)
    cst_e = nc.declare_dram_parameter("cst", [128, 8], FP32, isOutput=False)
    # bf16 consts: rows 0..3 = W_full in per-partition cols; rows 4..4+TT-1 =
    # per-token-tile low-batch masks; rows 4+TT.. = high-batch masks
    NCSTB = AT + 2 * TT + B32
    cstb_e = nc.declare_dram_parameter("cstb", [128, NCSTB], BF16, isOutput=False)
    awe_e = nc.declare_dram_parameter("awe", [B32, ENC], FP32, isOutput=True)
    alpha_e = nc.declare_dram_parameter("alpha", [B32, N], FP32, isOutput=True)

    with tile.TileContext(nc, trace_sim=trace_sim) as tc:
        with (
            tc.tile_pool(name="singles", bufs=1) as singles,
            tc.tile_pool(name="xtp", bufs=2) as xtp,
            tc.tile_pool(name="xnp", bufs=14) as xnp,
            tc.tile_pool(name="relup", bufs=2) as relup,
            tc.tile_pool(name="alphatp", bufs=6) as alphatp,
            tc.tile_pool(name="ps_att1", bufs=2, space="PSUM") as ps_att1,
            tc.tile_pool(name="ps_score", bufs=1, space="PSUM") as ps_score,
            tc.tile_pool(name="ps_small", bufs=1, space="PSUM") as ps_small,
            tc.tile_pool(name="ps_awe", bufs=1, space="PSUM") as ps_awe,
        ):
            # ---- constants / weights ----
            we_sb = singles.tile([128, KT, ATT], BF16)
            nc.sync.dma_start(out=we_sb, in_=we_e[:])
            wd_sb = singles.tile([128, DT, ATT], BF16)
            nc.sync.dma_start(out=wd_sb, in_=wd_e[:])
            dht_sb = singles.tile([128, DT, B32], BF16)
            nc.sync.dma_start(out=dht_sb, in_=dht_e[:])
            cst_sb = singles.tile([128, 8], FP32)
            nc.sync.dma_start(out=cst_sb, in_=cst_e[:])
            cstb_sb = singles.tile([128, NCSTB], BF16)
            nc.sync.dma_start(out=cstb_sb, in_=cstb_e[:])
            bias_sb = cst_sb[:, 0:AT]        # (b_enc + b_dec) per-partition, col a
            i1_sb = cst_sb[0:1, AT:AT + 1]   # 1.0 (identity for [1,n] transposes)

            # ---- long-lived rows / outputs ----
            exp_row = singles.tile([1, T], FP32)
            alpha_row = singles.tile([1, T], FP32)
            z_row = singles.tile([1, B32], FP32)
            zi_row = singles.tile([1, B32], FP32)
            awe_sb = singles.tile([B32, ENC], FP32)

            # ---- att2^T [a, b] + bias, once per core ----
            att2t_sb = singles.tile([128, AT, B32], FP32)
            for a in range(AT):
                ps = ps_small.tile([128, B32], FP32, tag="small")
                for k in range(DT):
                    nc.tensor.matmul(
                        ps,
                        wd_sb[:, k, a * 128:(a + 1) * 128],
                        dht_sb[:, k, :],
                        start=(k == 0),
                        stop=(k == DT - 1),
                    )
                nc.vector.tensor_scalar_add(att2t_sb[:, a, :], ps, bias_sb[:, a:a + 1])

            # ---- awe accumulators: 4 persistent PSUM banks [32, <=512] ----
            ECH = [512, 512, 512, 256]
            EOF_ = [0, 512, 1024, 1536]
            AWE_P = 128 if colt else B32
            awe_ps = [
                ps_awe.tile([AWE_P, ECH[j]], FP32, tag=f"awe{j}", name=f"awe_ps{j}")
                for j in range(4)
            ] if level >= 5 else []

            xn_tiles = {}
            emitted_softmax = set()
            emitted_awe = set()

            for c in range(nchunks):
                t_lo = c * CW
                cw = min(CW, T - t_lo)

                # loads for this chunk
                xt_sb = xtp.tile([128, KT, CW], BF16, tag="xt")
                nc.sync.dma_start(out=xt_sb, in_=xt_e[c])
                for tt in range(t_lo // 128, (t_lo + cw) // 128):
                    xn_t = xnp.tile([128, ENC], BF16, tag="xn")
                    nc.sync.dma_start(out=xn_t, in_=xn_e[tt * 128:(tt + 1) * 128, :])
                    xn_tiles[tt] = xn_t

                # att1^T + fused relu/bias evacuation
                relu_sb = relup.tile([128, AT, CW], BF16, tag="relu")
                for a in range(AT):
                    ps = ps_att1.tile([128, CW], FP32, tag="att1")
                    for k in range(KT):
                        nc.tensor.matmul(
                            ps[:, :cw],
                            we_sb[:, k, a * 128:(a + 1) * 128],
                            xt_sb[:, k, :cw],
                            start=(k == 0),
                            stop=(k == KT - 1),
                        )
                    if level < 1:
                        nc.scalar.activation(relu_sb[:, a, :cw], ps[:, :cw], AF.Relu)
                        continue
                    for (b, p0, p1) in _batch_pieces(t_lo, t_lo + cw):
                        nc.scalar.activation(
                            relu_sb[:, a, p0 - t_lo:p1 - t_lo],
                            ps[:, p0 - t_lo:p1 - t_lo],
                            AF.Relu,
                            bias=att2t_sb[:, a, b:b + 1],
                        )

                # scores for the chunk -> exp
                if level < 2:
                    continue
                sps = ps_score.tile([1, CW], FP32, tag="score")
                for a in range(AT):
                    nc.tensor.matmul(
                        sps[:, :cw],
                        cstb_sb[:, a:a + 1],
                        relu_sb[:, a, :cw],
                        start=(a == 0),
                        stop=(a == AT - 1),
                    )
                nc.scalar.activation(exp_row[0:1, t_lo:t_lo + cw], sps[0:1, :cw], AF.Exp)

                # softmax for batches whose tokens are now all scored
                for b in range(B32) if level >= 3 else []:
                    if b in emitted_softmax:
                        continue
                    if (b + 1) * N <= t_lo + cw:
                        nc.vector.reduce_sum(
                            out=z_row[0:1, b:b + 1],
                            in_=exp_row[0:1, b * N:(b + 1) * N],
                            axis=mybir.AxisListType.X,
                        )
                        nc.vector.reciprocal(zi_row[0:1, b:b + 1], z_row[0:1, b:b + 1])
                        nc.vector.tensor_scalar_mul(
                            alpha_row[0:1, b * N:(b + 1) * N],
                            exp_row[0:1, b * N:(b + 1) * N],
                            zi_row[0:1, b:b + 1],
                        )
                        emitted_softmax.add(b)

                # awe: emit per quad of 4 token-tiles, column-tiled so the
                # four matmuls run concurrently in distinct 32-col PE groups
                def _tile_ready(tt):
                    return tt in xn_tiles and ((tt * 128 + 127) // N) in emitted_softmax

                def _build_at(tt):
                    tp = ps_small.tile([128, 1], FP32, tag="small", name=f"tp_{tt}")
                    nc.tensor.transpose(tp, alpha_row[0:1, tt * 128:(tt + 1) * 128], i1_sb)
                    at_sb = alphatp.tile([128, B32], BF16, tag="alphat", name=f"at_{tt}")
                    nc.vector.memset(at_sb, 0.0)
                    pieces = _batch_pieces(tt * 128, (tt + 1) * 128)
                    b_lo = pieces[0][0]
                    nc.vector.tensor_mul(
                        at_sb[:, b_lo:b_lo + 1], tp, cstb_sb[:, AT + tt:AT + tt + 1]
                    )
                    if len(pieces) > 1:
                        b_hi = pieces[1][0]
                        nc.vector.tensor_mul(
                            at_sb[:, b_hi:b_hi + 1], tp,
                            cstb_sb[:, AT + TT + tt:AT + TT + tt + 1],
                        )
                    return at_sb

                NQ = (TT + 3) // 4
                for q in range(NQ) if level >= 4 else []:
                    if q in emitted_awe:
                        continue
                    tiles = list(range(4 * q, min(4 * q + 4, TT)))
                    if not all(_tile_ready(tt) for tt in tiles):
                        continue
                    if colt:
                        ats = [_build_at(tt) for tt in tiles]
                        for j in range(4) if level >= 5 else []:
                            for g, tt in enumerate(tiles):
                                nc.tensor.matmul(
                                    awe_ps[j][32 * g:32 * g + B32, :],
                                    ats[g],
                                    xn_tiles[tt][:, EOF_[j]:EOF_[j] + ECH[j]],
                                    start=(tt == 0),
                                    stop=(tt >= TT - 4),
                                    tile_position=(0, 32 * g),
                                    skip_group_check=True,
                                )
                    else:
                        for tt in tiles:
                            at_sb = _build_at(tt)
                            for j in range(4) if level >= 5 else []:
                                nc.tensor.matmul(
                                    awe_ps[j],
                                    at_sb,
                                    xn_tiles[tt][:, EOF_[j]:EOF_[j] + ECH[j]],
                                    start=(tt == 0),
                                    stop=(tt == TT - 1),
                                )
                    emitted_awe.add(q)

            # ---- epilogue: evacuate awe (fold col-groups), store outputs ----
            bones_sb = cstb_sb[:, AT + 2 * TT:AT + 2 * TT + B32]
            for j in range(4) if level >= 5 else []:
                if colt:
                    p_sb = singles.tile([128, ECH[j]], BF16, name=f"p_sb{j}")
                    nc.scalar.activation(p_sb, awe_ps[j], AF.Copy)
                    fold = ps_small.tile([B32, ECH[j]], FP32, tag="small", name=f"fold{j}")
                    nc.tensor.matmul(fold, bones_sb, p_sb, start=True, stop=True)
                    nc.scalar.activation(awe_sb[:, EOF_[j]:EOF_[j] + ECH[j]], fold, AF.Copy)
                else:
                    nc.scalar.activation(awe_sb[:, EOF_[j]:EOF_[j] + ECH[j]], awe_ps[j], AF.Copy)
            if level < 5:
                nc.vector.memset(awe_sb, 0.0)
            nc.sync.dma_start(out=awe_e[:], in_=awe_sb)
            if level < 3:
                nc.vector.memset(alpha_row, 0.0)
            nc.sync.dma_start(out=alpha_e.rearrange("b n -> (b n)"), in_=alpha_row[0:1, :])

    nc.compile()
    return nc


_NC_CACHE = None


def _get_nc():
    global _NC_CACHE
    if _NC_CACHE is None:
        _NC_CACHE = build()
    return _NC_CACHE


def make_in_maps(encoder_out, decoder_hidden, W_enc, b_enc, W_dec, b_dec, W_full, b_full):
    enc = np.ascontiguousarray(np.asarray(encoder_out, dtype=np.float32))
    dh = np.asarray(decoder_hidden, dtype=np.float32)

    # per-partition pre-rearranged weights: [p, k, a] with rows of dim k*128+p
    we_b = np.ascontiguousarray(
        np.asarray(W_enc, np.float32).reshape(KT, 128, ATT).transpose(1, 0, 2)
    ).astype(bf16)
    wd_b = np.ascontiguousarray(
        np.asarray(W_dec, np.float32).reshape(DT, 128, ATT).transpose(1, 0, 2)
    ).astype(bf16)
    cstb = np.zeros((AT + 2 * TT, 128), np.float32)
    cstb[0:AT] = np.asarray(W_full, dtype=np.float32).reshape(AT, 128)
    for tt in range(TT):
        b_lo = (tt * 128) // N
        r_split = min(128, (b_lo + 1) * N - tt * 128)
        cstb[AT + tt, :r_split] = 1.0
        cstb[AT + TT + tt, r_split:] = 1.0
    cstb = np.ascontiguousarray(cstb.T).astype(bf16)
    cst = np.zeros((8, 128), np.float32)
    cst[0:AT] = (np.asarray(b_enc, np.float32) + np.asarray(b_dec, np.float32)).reshape(AT, 128)
    cst[AT, 0] = 1.0  # identity scalar for PE row transposes
    cst = np.ascontiguousarray(cst.T)

    enc_b = enc.reshape(B, N, ENC).astype(bf16)
    in_maps = []
    for c in range(NCORES):
        xn = np.ascontiguousarray(enc_b[c * B32:(c + 1) * B32].reshape(T, ENC))
        # chunk-contiguous transposed layout: xt[c, p, k, t] = X^T[k*128+p, CW*c+t]
        xn_pad = np.zeros((NCHUNK * CW, ENC), bf16)
        xn_pad[:T] = xn
        xt = np.ascontiguousarray(
            xn_pad.reshape(NCHUNK, CW, KT, 128).transpose(0, 3, 2, 1)
        )
        dht = np.ascontiguousarray(
            dh[c * B32:(c + 1) * B32].T.reshape(DT, 128, B32).transpose(1, 0, 2).astype(bf16)
        )
        in_maps.append({
            "xt": xt, "xn": xn, "we": we_b, "wd": wd_b, "dht": dht,
            "cst": cst, "cstb": cstb,
        })
    return in_maps


def kernel(encoder_out, decoder_hidden, W_enc, b_enc, W_dec, b_dec, W_full, b_full):
    from concourse.bass_utils import run_bass_kernel_spmd

    nc = _get_nc()
    in_maps = make_in_maps(encoder_out, decoder_hidden, W_enc, b_enc, W_dec, b_dec,
                           W_full, b_full)
    res = run_bass_kernel_spmd(nc, in_maps, list(range(NCORES)))
    awe = np.concatenate([res.results[c]["awe"] for c in range(NCORES)], axis=0)
    alpha = np.concatenate([res.results[c]["alpha"] for c in range(NCORES)], axis=0)
    return awe, alpha
